# revision 3
# baseline (speedup 1.0000x reference)
"""Chamfer loss kernel for trn2 (8 NeuronCores).

Problem: preds [4, 8192, 3], gts [4, 8192, 3] (f32).
  P[b,i,j] = ||gts[b,i] - preds[b,j]||^2
  loss = sum_j min_i P + sum_i min_j P   (scalar f32)

Sharding: 8 cores = (batch b in 0..3) x (pred-half h in 0..1).
Each core computes d^2 for its 8192 x 4096 block via fp32 matmuls with
K=5 augmented vectors (gx,gy,gz,||g||^2,1) x (-2px,-2py,-2pz,1,||p||^2),
then min-reduces on-chip in both directions; host combines partials.
"""

import sys

sys.path.insert(0, "/opt/trn_rl_repo")

import numpy as np

import bass_rust
import concourse.bass as bass
import concourse.tile as tile_mod
from concourse import mybir
from concourse.bass_utils import run_bass_kernel_spmd

N_CORES = 8
NI = 8192          # gts points per batch
NJ = 4096          # preds points per core (half batch)
ITILES = NI // 128  # 64 i-tiles of 128
NCHUNK = 2048      # j-chunk processed per DVE op (4 PSUM banks)
CHUNKS = NJ // NCHUNK  # 2

_NC_CACHE = None


def _split_waits(nc):
    # Workaround: this walrus build rejects instructions carrying more than
    # one sync wait ("Too many sync wait commands"). Move all but the last
    # wait onto single-wait NoOps inserted just before, on the same engine
    # queue (per-engine FIFO keeps the semantics identical).
    ctr = 0
    for f in nc.m.functions:
        for bb in f.blocks:
            new_insts = []
            changed = False
            for ins in bb.instructions:
                si = ins.sync_info
                waits = list(si.on_wait) if si is not None else []
                if len(waits) > 1:
                    changed = True
                    for w in waits[:-1]:
                        nop = mybir.InstNoOp(name=f"splitw_{ctr}", ins=[], outs=[])
                        ctr += 1
                        nop.engine = ins.engine
                        nop.sync_info = bass_rust.SyncInfo(
                            on_wait=[w], on_update=[]
                        )
                        new_insts.append(nop)
                    ins.sync_info = bass_rust.SyncInfo(
                        on_wait=[waits[-1]], on_update=list(si.on_update)
                    )
                new_insts.append(ins)
            if changed:
                bb.instructions = new_insts


def _build_nc():
    f32 = mybir.dt.float32
    nc = bass.Bass("TRN2", target_bir_lowering=False, debug=False)
    ga_d = nc.dram_tensor("ga", [5, NI], f32, kind="ExternalInput")
    pa_d = nc.dram_tensor("pa", [5, NJ], f32, kind="ExternalInput")
    rp_d = nc.dram_tensor("rp", [128, ITILES * CHUNKS], f32, kind="ExternalOutput")
    cm_d = nc.dram_tensor("cm", [128, NJ], f32, kind="ExternalOutput")

    with tile_mod.TileContext(nc) as tc:
        with (
            tc.tile_pool(name="const", bufs=1) as const_pool,
            tc.tile_pool(name="psum", bufs=2, space="PSUM") as psum_pool,
        ):
            ga_s = const_pool.tile([5, NI], f32)
            nc.gpsimd.dma_start(out=ga_s[:], in_=ga_d[:])
            pa_s = const_pool.tile([5, NJ], f32)
            nc.gpsimd.dma_start(out=pa_s[:], in_=pa_d[:])

            a_s = const_pool.tile([128, NJ], f32)   # colmin accumulator
            nc.vector.memset(a_s[:], 3.0e38)
            rp_s = const_pool.tile([128, ITILES * CHUNKS], f32)

            for it in range(ITILES):
                lhs = ga_s[:, it * 128:(it + 1) * 128]
                for ch in range(CHUNKS):
                    q = psum_pool.tile([128, NCHUNK], f32, tag="q")
                    for qb in range(NCHUNK // 512):
                        j0 = ch * NCHUNK + qb * 512
                        nc.tensor.matmul(
                            q[:, qb * 512:(qb + 1) * 512],
                            lhs,
                            pa_s[:, j0:j0 + 512],
                            start=True,
                            stop=True,
                        )
                    # row direction: min over j-chunk for each i (partition)
                    col = it * CHUNKS + ch
                    nc.vector.tensor_reduce(
                        rp_s[:, col:col + 1],
                        q[:],
                        mybir.AxisListType.X,
                        mybir.AluOpType.min,
                    )
                    # col direction: elementwise min accumulate over i-tiles
                    asl = a_s[:, ch * NCHUNK:(ch + 1) * NCHUNK]
                    nc.vector.tensor_tensor(asl, q[:], asl, mybir.AluOpType.min)

            nc.gpsimd.dma_start(out=rp_d[:], in_=rp_s[:])
            nc.gpsimd.dma_start(out=cm_d[:], in_=a_s[:])
    _split_waits(nc)
    return nc


def _get_nc():
    global _NC_CACHE
    if _NC_CACHE is None:
        _NC_CACHE = _build_nc()
    return _NC_CACHE


def _prep_inputs(preds, gts):
    in_maps = []
    for c in range(N_CORES):
        b, h = divmod(c, 2)
        g = gts[b]                            # [8192, 3]
        p = preds[b, h * NJ:(h + 1) * NJ]     # [4096, 3]
        ga = np.empty((5, NI), np.float32)
        ga[0:3] = g.T
        ga[3] = (g * g).sum(1)
        ga[4] = 1.0
        pa = np.empty((5, NJ), np.float32)
        pa[0:3] = -2.0 * p.T
        pa[3] = 1.0
        pa[4] = (p * p).sum(1)
        in_maps.append({"ga": np.ascontiguousarray(ga), "pa": np.ascontiguousarray(pa)})
    return in_maps


def _combine(results):
    loss = 0.0
    for b in range(4):
        rowmin = None
        for h in range(2):
            r = results[2 * b + h]
            rp = np.asarray(r["rp"], np.float64)          # [128, it*2+ch]
            rm = rp.reshape(128, ITILES, CHUNKS).min(2)   # [p, it]
            rm = rm.T.reshape(NI)                         # i = it*128 + p
            rowmin = rm if rowmin is None else np.minimum(rowmin, rm)
            cm = np.asarray(r["cm"], np.float64)          # [128, 4096]
            loss += cm.min(0).sum()
        loss += rowmin.sum()
    return np.float32(loss)


def kernel(preds, gts):
    preds = np.ascontiguousarray(np.asarray(preds, dtype=np.float32))
    gts = np.ascontiguousarray(np.asarray(gts, dtype=np.float32))
    assert preds.shape == (4, NI, 3) and gts.shape == (4, NI, 3)
    nc = _get_nc()
    in_maps = _prep_inputs(preds, gts)
    res = run_bass_kernel_spmd(nc, in_maps, list(range(N_CORES)))
    return _combine(res.results)


# revision 5
# speedup vs baseline: 1.4748x; 1.4748x over previous
"""Chamfer loss kernel for trn2 (8 NeuronCores).

Problem: preds [4, 8192, 3], gts [4, 8192, 3] (f32).
  P[b,i,j] = ||gts[b,i] - preds[b,j]||^2
  loss = sum_j min_i P + sum_i min_j P   (scalar f32)

Sharding: 8 cores = (batch b in 0..3) x (pred-half h in 0..1).
Each core computes d^2 for its 8192 x 4096 block via fp32 matmuls with
K=5 augmented vectors (gx,gy,gz,||g||^2,1) x (-2px,-2py,-2pz,1,||p||^2),
then min-reduces on-chip in both directions; host combines partials.
"""

import sys

sys.path.insert(0, "/opt/trn_rl_repo")

import numpy as np

import bass_rust
import concourse.bass as bass
import concourse.tile as tile_mod
from concourse import mybir
from concourse.bass_utils import run_bass_kernel_spmd

N_CORES = 8
NI = 8192          # gts points per batch
NJ = 4096          # preds points per core (half batch)
ITILES = NI // 128  # 64 i-tiles of 128
NCHUNK = 2048      # j-chunk processed per DVE op (4 PSUM banks)
CHUNKS = NJ // NCHUNK  # 2

_NC_CACHE = None


def _split_waits(nc):
    # Workaround: this walrus build rejects instructions carrying more than
    # one sync wait ("Too many sync wait commands"). Move all but the last
    # wait onto single-wait NoOps inserted just before, on the same engine
    # queue (per-engine FIFO keeps the semantics identical).
    ctr = 0
    for f in nc.m.functions:
        for bb in f.blocks:
            new_insts = []
            changed = False
            for ins in bb.instructions:
                si = ins.sync_info
                waits = list(si.on_wait) if si is not None else []
                if len(waits) > 1:
                    changed = True
                    for w in waits[:-1]:
                        nop = mybir.InstNoOp(name=f"splitw_{ctr}", ins=[], outs=[])
                        ctr += 1
                        nop.engine = ins.engine
                        nop.sync_info = bass_rust.SyncInfo(
                            on_wait=[w], on_update=[]
                        )
                        new_insts.append(nop)
                    ins.sync_info = bass_rust.SyncInfo(
                        on_wait=[waits[-1]], on_update=list(si.on_update)
                    )
                new_insts.append(ins)
            if changed:
                bb.instructions = new_insts


def _build_nc():
    f32 = mybir.dt.float32
    nc = bass.Bass("TRN2", target_bir_lowering=False, debug=False)
    ga_d = nc.dram_tensor("ga", [5, NI], f32, kind="ExternalInput")
    pa_d = nc.dram_tensor("pa", [5, NJ], f32, kind="ExternalInput")
    rp_d = nc.dram_tensor("rp", [128, ITILES * CHUNKS], f32, kind="ExternalOutput")
    cm_d = nc.dram_tensor("cm", [128, NJ], f32, kind="ExternalOutput")

    with tile_mod.TileContext(nc) as tc:
        with (
            tc.tile_pool(name="const", bufs=1) as const_pool,
            tc.tile_pool(name="evac", bufs=4) as evac_pool,
            tc.tile_pool(name="psum", bufs=2, space="PSUM") as psum_pool,
        ):
            # lhs/rhs replicated at partition bases 0/32/64/96 so four K=5
            # matmuls run concurrently in distinct PE row groups.
            ga_s = const_pool.tile([101, NI], f32)
            pa_s = const_pool.tile([101, NJ], f32)
            for r in range(4):
                nc.gpsimd.dma_start(out=ga_s[32 * r:32 * r + 5, :], in_=ga_d[:])
                nc.gpsimd.dma_start(out=pa_s[32 * r:32 * r + 5, :], in_=pa_d[:])

            a_s = const_pool.tile([128, NJ], f32)   # colmin accumulator
            nc.vector.memset(a_s[:], 3.0e38)
            rp_s = const_pool.tile([128, ITILES * CHUNKS], f32)
            s_s = const_pool.tile([128, NCHUNK], f32)  # tensor_scalar dump

            for it in range(ITILES):
                for ch in range(CHUNKS):
                    q = psum_pool.tile([128, NCHUNK], f32, tag="q")
                    for r in range(4):
                        j0 = ch * NCHUNK + r * 512
                        nc.tensor.matmul(
                            q[:, r * 512:(r + 1) * 512],
                            ga_s[32 * r:32 * r + 5, it * 128:(it + 1) * 128],
                            pa_s[32 * r:32 * r + 5, j0:j0 + 512],
                            start=True,
                            stop=True,
                            tile_position=(32 * r, 0),
                        )
                    # evacuate PSUM -> SBUF on ACT (sole PSUM reader)
                    c = evac_pool.tile([128, NCHUNK], f32, tag="c")
                    nc.scalar.copy(c[:], q[:])
                    # row direction: min over j-chunk per i, via single-src
                    # tensor_scalar (accum reduce op1=min); out dumped to s_s
                    col = it * CHUNKS + ch
                    nc.vector.tensor_scalar(
                        s_s[:],
                        c[:],
                        1.0,
                        None,
                        mybir.AluOpType.mult,
                        mybir.AluOpType.min,
                        accum_out=rp_s[:, col:col + 1],
                    )
                    # col direction: elementwise min accumulate over i-tiles
                    # (GPSIMD TensorTensor is rejected by this walrus build,
                    # so both chunk chains run on DVE)
                    asl = a_s[:, ch * NCHUNK:(ch + 1) * NCHUNK]
                    nc.vector.tensor_tensor(asl, c[:], asl, mybir.AluOpType.min)

            nc.gpsimd.dma_start(out=rp_d[:], in_=rp_s[:])
            nc.gpsimd.dma_start(out=cm_d[:], in_=a_s[:])
    _split_waits(nc)
    return nc


def _get_nc():
    global _NC_CACHE
    if _NC_CACHE is None:
        _NC_CACHE = _build_nc()
    return _NC_CACHE


def _prep_inputs(preds, gts):
    in_maps = []
    for c in range(N_CORES):
        b, h = divmod(c, 2)
        g = gts[b]                            # [8192, 3]
        p = preds[b, h * NJ:(h + 1) * NJ]     # [4096, 3]
        ga = np.empty((5, NI), np.float32)
        ga[0:3] = g.T
        ga[3] = (g * g).sum(1)
        ga[4] = 1.0
        pa = np.empty((5, NJ), np.float32)
        pa[0:3] = -2.0 * p.T
        pa[3] = 1.0
        pa[4] = (p * p).sum(1)
        in_maps.append({"ga": np.ascontiguousarray(ga), "pa": np.ascontiguousarray(pa)})
    return in_maps


def _combine(results):
    loss = 0.0
    for b in range(4):
        rowmin = None
        for h in range(2):
            r = results[2 * b + h]
            rp = np.asarray(r["rp"], np.float64)          # [128, it*2+ch]
            rm = rp.reshape(128, ITILES, CHUNKS).min(2)   # [p, it]
            rm = rm.T.reshape(NI)                         # i = it*128 + p
            rowmin = rm if rowmin is None else np.minimum(rowmin, rm)
            cm = np.asarray(r["cm"], np.float64)          # [128, 4096]
            loss += cm.min(0).sum()
        loss += rowmin.sum()
    return np.float32(loss)


def kernel(preds, gts):
    preds = np.ascontiguousarray(np.asarray(preds, dtype=np.float32))
    gts = np.ascontiguousarray(np.asarray(gts, dtype=np.float32))
    assert preds.shape == (4, NI, 3) and gts.shape == (4, NI, 3)
    nc = _get_nc()
    in_maps = _prep_inputs(preds, gts)
    res = run_bass_kernel_spmd(nc, in_maps, list(range(N_CORES)))
    return _combine(res.results)
